# revision 36
# baseline (speedup 1.0000x reference)
"""Binary-conv BasicBlock (sign-act 3x3 binary conv + BN(eval) + residual).

Full shapes: x (32,128,56,56) f32, weight (128,128,3,3), BN params (128,).
Strategy: data-parallel over batch N across 8 NeuronCores (4 images/core).

Per image on-device (fp8 DoubleRow formulation — HW-benched: DR matmuls
stream at 1 column/cycle with 256-deep contraction, s2s = N/2.4GHz + 2.5ns):
  - sign(x) on ScalarE into a zero-framed fp8 tile with 64-wide rows
    (58 rows; cols 57..63 junk-zero).  A second "slot" holds the same
    rows shifted left by 2 cols (VectorE copy), so kh=2's kw=0/kw=2 taps
    pair into one DR matmul via the inter-slot j-step (3712B, %16==0).
  - conv per 7-row chunk = 5 matmuls streaming 448 cols each (full
    64-wide rows; kw shift folded into the rhs offset so all taps land
    on the same psum grid; cols 56..63 of each row-block are junk):
      3x DR (kh0+kh1 pairs @ kw0/1/2, j-step 64B)
      1x DR (kh2: kw0 + kw2-via-slot1, j-step 3712B)
      1x plain (kh2 @ kw1)
    = 2240 streamed cols/chunk vs 3528 for the 9-tap bf16 version.
  - epilogue on VectorE: out = (psum * s) + (x + t) via
    scalar_tensor_tensor, strided psum read (64,7)x(1,56), bf16 out
    (stores halve; bf16 quantization ~0.3% << 2e-2 tolerance).
    The input is pre-biased on the host (xp = x + t) so the residual
    reads the input tile directly; signs recover sign(x) via ACT
    bias=-t.  ScalarE per image = 3 signs + 2 tick separators, under
    the 7.57us PE window.
  - startup: every dma_start costs ~600ns descriptor-gen and the two
    DGE queues share the 16 SDMA engines (~257GB/s sustained total),
    so startup loads all ride the sync queue, w first; steady-state
    x loads go via the scalar queue, stores via sync.  40 warmup
    matmuls bridge preamble-end to first-deps-ready (a PE idle gap
    resets the HAM clock ramp to 1.2GHz for ~3us - measured).
"""

import numpy as np
import ml_dtypes

_N, _C, _H, _W = 32, 128, 56, 56
_P = 128
_NCORES = 8
_NPI = _N // _NCORES  # images per core
_WP = 64              # padded fp8 row width (j-step 64B)
_HP = _H + 2          # 58 rows
_SLOT = _HP * _WP     # 3712 B/partition per slot
_NPIX = _H * _W
_BN_EPS = 1e-5
_CH = 7               # output rows per PSUM bank chunk
_NCH = _H // _CH      # 8 chunks per image
_NPAIR = _NCH // 2    # 4 psum pair-tiles (2 banks each) per image
_CN = _CH * _W        # 392 valid elems per chunk
_CS = _CH * _WP       # 448 streamed columns per chunk

_cache = {}


def _build_program():
    import concourse.bass as bass
    import concourse.bacc as bacc
    import concourse.mybir as mybir
    import concourse.tile as tile

    f32 = mybir.dt.float32
    bf16 = mybir.dt.bfloat16
    fp8 = mybir.dt.float8e4
    DR = mybir.MatmulPerfMode.DoubleRow

    nc = bacc.Bacc("TRN2", target_bir_lowering=False, debug=False)

    # "x" is pre-biased on the host: xp = x + t (t = BN shift, per channel).
    # The epilogue residual needs (x + t) anyway, and the signs recover
    # sign(x) via the ACT bias (-t) — this deletes the whole per-image
    # xp=x+t ScalarE pass (~1.9us/image, Scalar was the bottleneck).
    x_d = nc.dram_tensor("x", [_NPI, _C, _NPIX], f32, kind="ExternalInput")
    w_d = nc.dram_tensor("w", [_C, 9, _P], fp8, kind="ExternalInput")
    st_d = nc.dram_tensor("st", [_P, 2], f32, kind="ExternalInput")
    o_d = nc.dram_tensor("o", [_NPI, _P, _NPIX], bf16, kind="ExternalOutput")

    SIGN = mybir.ActivationFunctionType.Sign
    IDENT = mybir.ActivationFunctionType.Identity
    MULT, ADD = mybir.AluOpType.mult, mybir.AluOpType.add

    with tile.TileContext(nc) as tc:
        with (
            tc.tile_pool(name="const", bufs=1) as cpool,
            tc.tile_pool(name="xin", bufs=4) as xpool,
            tc.tile_pool(name="apad", bufs=1) as apool,
            tc.tile_pool(name="outp", bufs=6) as opool,
            tc.tile_pool(name="ps", bufs=4, space="PSUM") as pspool,
        ):
            # Warmup source on GpSimd (its preamble ends ~1.2us before
            # Vector's, so warmups start right at Tensor preamble end).
            dummy = cpool.tile([_C, _P], fp8)
            nc.gpsimd.memset(dummy[:], 0.0)
            # Throwaway Sign so the 1.3us ACT_TABLE_LOAD runs during the
            # initial DMA wait.
            scratch = cpool.tile([_C, 8], fp8)
            nc.scalar.sign(scratch[:], dummy[:, 0:8])

            x_tiles = [None] * _NPI

            def load_x(n, ranges, eng=None):
                """x loads; images 2-3 go via the Scalar DGE queue so input
                transfers don't serialize behind output stores on Sync (the
                single queue sustains only ~257GB/s, measured)."""
                eng = eng or nc.sync
                if x_tiles[n] is not None:
                    x_t = x_tiles[n]
                else:
                    x_t = xpool.tile([_C, _NPIX], f32, name="x_t", tag="x")
                    x_tiles[n] = x_t
                for r0, r1 in ranges:
                    eng.dma_start(
                        x_t[:, r0 * _W : r1 * _W],
                        x_d[n, :, r0 * _W : r1 * _W],
                    )

            IMG0_RANGES = [(0, 8), (8, 22), (22, 42), (42, 56)]

            # All startup loads on the sync queue, w first (it gates the
            # first real matmul's LDW): the two DGE queues SHARE the 16
            # SDMA engines, so splitting startup transfers across queues
            # just halves each one's rate (measured: +3.4us on mm0).
            wt = cpool.tile([_C, 9, _P], fp8)
            nc.sync.dma_start(wt[:], w_d[:])
            load_x(0, IMG0_RANGES[:1])
            st_t = cpool.tile([_P, 2], f32)
            nc.sync.dma_start(st_t[:], st_d[:])
            load_x(0, IMG0_RANGES[1:])
            s_ap = st_t[:, 0:1]
            nt_ap = st_t[:, 1:2]  # -t: sign(x) = Sign(xp + (-t))

            # Two persistent double-slot fp8 sign tiles (ping-pong across
            # images).  Only the frame/junk cells are zeroed (once):
            #   slot0: top row, bottom row, col 0, cols 57..63
            #   slot1: rows 0..1 (wrap-read slack), row 57, cols 56..63
            a_tiles = []
            for i in range(2):
                a_t = apool.tile([_C, 2 * _SLOT], fp8, name=f"apad{i}", tag=f"apad{i}")
                part = tuple(a_t[:, 0:1].ap[0])
                base = int(a_t[:, 0:1].offset)

                def ap_of(off, dims):
                    return bass.AP(tensor=a_t.tensor, offset=base + off, ap=[part] + dims)

                nc.vector.memset(a_t[:, 0:_WP], 0.0)                      # s0 top
                nc.vector.memset(a_t[:, 57 * _WP : _SLOT], 0.0)           # s0 bottom
                nc.vector.memset(ap_of(0, [(_WP, _HP), (1, 1)]), 0.0)     # s0 col0
                nc.vector.memset(ap_of(57, [(_WP, _HP), (1, 7)]), 0.0)    # s0 c57-63
                nc.vector.memset(a_t[:, _SLOT : _SLOT + 2 * _WP], 0.0)    # s1 r0-1
                nc.vector.memset(a_t[:, _SLOT + 57 * _WP :], 0.0)         # s1 r57
                nc.vector.memset(
                    ap_of(_SLOT + 55, [(_WP, _HP), (1, 9)]), 0.0          # s1 c55-63
                )
                a_tiles.append(a_t)

            def sign_slice(n, r0, r1):
                """ScalarE: sign(x rows r0..r1) -> slot0 rows 1+r0..1+r1."""
                x_v = x_tiles[n][:].rearrange("c (h w) -> c h w", h=_H)
                a_v = a_tiles[n % 2][:, 0:_SLOT].rearrange("c (h w) -> c h w", w=_WP)
                nc.scalar.activation(
                    a_v[:, 1 + r0 : 1 + r1, 1 : _W + 1],
                    x_v[:, r0:r1, :],
                    SIGN,
                    bias=nt_ap,
                )

            def sign2_slice(n, r0, r1):
                """ScalarE: slot1[r, u] = a[r, u+2] = sign(x[r-1, u+1]) — the
                shift-by-2 copy computed directly from x (keeps VectorE free
                for epilogues).  x rows r0..r1 -> slot1 rows 1+r0..1+r1,
                cols 0..54 (col 55 is the a-col-57 frame zero, pre-memset)."""
                x_v = x_tiles[n][:].rearrange("c (h w) -> c h w", h=_H)
                a1_v = a_tiles[n % 2][:, _SLOT:].rearrange("c (h w) -> c h w", w=_WP)
                nc.scalar.activation(
                    a1_v[:, 1 + r0 : 1 + r1, 0:55],
                    x_v[:, r0:r1, 1:56],
                    SIGN,
                    bias=nt_ap,
                )

            # Tiny ScalarE op: dependency "tick" separator.  Dependents of
            # the op before it release ~60ns after it instead of waiting
            # through the next full-size ACT (the sem-tick granularity
            # otherwise rounds the release up to the next op's end).
            tick_t = cpool.tile([_C, 8], bf16)

            def tick():
                nc.scalar.activation(tick_t[:], dummy[:, 0:8], IDENT)

            # Image-0 staging: signs only (residual uses the pre-biased
            # input tile directly; image 0 uses the 6-mm no-slot1 form).
            sign_slice(0, *IMG0_RANGES[0])
            tick()
            sign_slice(0, *IMG0_RANGES[1])
            tick()
            sign_slice(0, *IMG0_RANGES[2])
            tick()
            sign_slice(0, *IMG0_RANGES[3])
            tick()

            def stage_sign(n, ranges):
                """ScalarE staging for images 1..3.  sign2 only for the first
                half: the second half's slot1 rows come from a VectorE copy
                (stage_copy) so Scalar stays under the per-image budget."""
                (a0, a1), (b0, b1) = ranges
                sign_slice(n, a0, a1)
                sign2_slice(n, max(a0, 1), a1)
                tick()
                sign_slice(n, b0, b1)
                tick()

            def stage_copy(n):
                """VectorE: slot1 rows 29..57 = slot0 rows 29..57 shifted by
                2 cols (row 57 reads the frame zeros).  Emitted at the END of
                the previous image's pair loop so it can't head-block that
                image's epilogue stts."""
                a_t = a_tiles[n % 2]
                av = a_t[:].rearrange("c (s h w) -> c s h w", s=2, w=_WP)
                nc.vector.tensor_copy(
                    av[:, 1, 29:58, 0:56],
                    av[:, 0, 29:58, 2:58],
                )

            # PE warmup (results discarded).  ~107ns apart at mid-clock;
            # covers preamble-end -> first-deps-ready (~10.8us: sign0 after
            # the startup DGE chain).  A PE idle gap here resets the HAM
            # clock ramp, so over-covering beats under-covering.
            warm_ps = pspool.tile([_P, 2, 512], f32, name="warm_ps", tag="ps")
            for i in range(64):
                nc.tensor.matmul(
                    warm_ps[:, i % 2, :128],
                    dummy[:],
                    dummy[:],
                    start=True,
                    stop=True,
                )

            def chunk_mms(n, bank_ap, rb, use_slot1):
                """Matmuls for the chunk at a-row rb (out rows rb..rb+6),
                bank-major (tap-major pairs were tried: steady state gained
                only ~35ns/image but image-0's pair-0 then gates on sign1 —
                net loss).
                use_slot1: 5-mm form (needs slot1 copies done); else 6-mm
                form with kh=2 as three plain taps (image 0: avoids coupling
                the startup-critical first chunks to the copy pipeline)."""
                a_t = a_tiles[n % 2]
                part = tuple(a_t[:, 0:1].ap[0])
                base = int(a_t[:, 0:1].offset)

                def rhs(off, step):
                    dims = [part]
                    if step is not None:
                        dims.append((step, 2))
                    dims.append((1, _CS))
                    return bass.AP(tensor=a_t.tensor, offset=base + off, ap=dims)

                plan = [
                    (wt[:, 0:2, :], rb * _WP + 0, _WP, DR),
                    (wt[:, 2:4, :], rb * _WP + 1, _WP, DR),
                    (wt[:, 4:6, :], rb * _WP + 2, _WP, DR),
                ]
                if use_slot1:
                    plan += [
                        (wt[:, 6:8, :], (rb + 2) * _WP + 0, _SLOT, DR),
                        (wt[:, 8, :], (rb + 2) * _WP + 1, None, None),
                    ]
                else:
                    plan += [
                        (wt[:, 6, :], (rb + 2) * _WP + 0, None, None),
                        (wt[:, 8, :], (rb + 2) * _WP + 1, None, None),
                        (wt[:, 7, :], (rb + 2) * _WP + 2, None, None),
                    ]
                for i, (w_ap, off, step, pm) in enumerate(plan):
                    nc.tensor.matmul(
                        bank_ap,
                        w_ap,
                        rhs(off, step),
                        start=(i == 0),
                        stop=(i == len(plan) - 1),
                        perf_mode=pm,
                    )

            for n in range(_NPI):
                if n + 1 < _NPI:
                    # image 1's loads on sync (startup transfers still
                    # draining there; a second queue would steal SDMA
                    # engines); images 2-3 via the scalar queue.
                    load_x(n + 1, [(0, 28), (28, 56)],
                           eng=(nc.sync if n == 0 else nc.scalar))
                    stage_sign(n + 1, [(0, 28), (28, 56)])

                last_img = n == _NPI - 1

                for p in range(_NPAIR):
                    fine_tail = last_img and p == _NPAIR - 1
                    if fine_tail:
                        bank_tiles = [
                            pspool.tile([_P, 512], f32, name=f"pstb{b}", tag="ps")
                            for b in range(2)
                        ]
                        bank_aps = [bt[:, :_CS] for bt in bank_tiles]
                        bank_views = [
                            bt[:].rearrange("c (h w) -> c h w", w=_WP)
                            for bt in bank_tiles
                        ]
                    else:
                        pst = pspool.tile([_P, 2, 512], f32, name="pst", tag="ps")
                        bank_aps = [pst[:, b, :_CS] for b in range(2)]
                        pv = pst[:].rearrange("c b (h w) -> c b h w", w=_WP)
                        bank_views = [pv[:, b] for b in range(2)]
                    out_t = opool.tile([_P, 2 * _CN], bf16, name="out_t", tag="o")

                    def epi(b, h0, h1):
                        """stt halves/full of bank b into out_t: out =
                        psum*s + (x+t), the residual read straight from the
                        pre-biased input tile."""
                        nc.vector.scalar_tensor_tensor(
                            out_t[:, b * _CN + h0 * _W : b * _CN + h1 * _W],
                            bank_views[b][:, h0:h1, 0:56],
                            s_ap,
                            x_tiles[n][
                                :, (2 * p + b) * _CN + h0 * _W :
                            ][:, : (h1 - h0) * _W],
                            MULT,
                            ADD,
                        )

                    for b in range(2):
                        chunk_mms(n, bank_aps[b], (2 * p + b) * _CH,
                                  use_slot1=(n > 0))
                        if fine_tail:
                            c0 = (2 * p + b) * _CN
                            if b == 0:
                                # bank 0's store DGE rides the (idle) scalar
                                # queue so the three tail store issues don't
                                # serialize ~1.8us on sync
                                epi(b, 0, _CH)
                                nc.scalar.dma_start(
                                    o_d[n, :, c0 : c0 + _CN],
                                    out_t[:, 0:_CN],
                                )
                            else:
                                for h0, h1 in ((0, 4), (4, _CH)):
                                    epi(b, h0, h1)
                                    nc.sync.dma_start(
                                        o_d[n, :, c0 + h0 * _W : c0 + h1 * _W],
                                        out_t[
                                            :, _CN + h0 * _W : _CN + h1 * _W
                                        ],
                                    )
                    if not fine_tail:
                        # per-bank stts (walrus limits STT inputs to 2D/3D
                        # APs, so the two banks can't merge into one 4D op)
                        for b in range(2):
                            epi(b, 0, _CH)
                        nc.sync.dma_start(
                            o_d[n, :, p * 2 * _CN : (p + 1) * 2 * _CN],
                            out_t[:],
                        )
                        if p == _NPAIR - 1 and n + 1 < _NPI:
                            stage_copy(n + 1)

    nc.compile()
    return nc


def _get_program():
    if "nc" not in _cache:
        _cache["nc"] = _build_program()
    return _cache["nc"]


def _prep_inputs(x, weight, bias, gamma, beta, running_mean, running_var):
    # per-core batch shards
    xs = np.ascontiguousarray(
        np.asarray(x, dtype=np.float32).reshape(_NCORES, _NPI, _C, _NPIX)
    )
    # sign(weight) packed as [C, k, P] fp8, k-order =
    # [(0,0),(1,0),(0,1),(1,1),(0,2),(1,2),(2,0),(2,2),(2,1)] (kh,kw)
    wb = np.sign(np.asarray(weight, dtype=np.float32))  # [P, C, 3, 3]
    korder = [(0, 0), (1, 0), (0, 1), (1, 1), (0, 2), (1, 2), (2, 0), (2, 2), (2, 1)]
    wT = np.stack(
        [wb[:, :, kh, kw].T for kh, kw in korder], axis=1
    )  # [C, 9, P]
    wT = np.ascontiguousarray(wT).astype(ml_dtypes.float8_e4m3)
    inv = np.asarray(gamma, dtype=np.float64) / np.sqrt(
        np.asarray(running_var, dtype=np.float64) + _BN_EPS
    )
    shift = (
        np.asarray(bias, dtype=np.float64) * inv
        + np.asarray(beta, dtype=np.float64)
        - np.asarray(running_mean, dtype=np.float64) * inv
    )
    t32 = shift.astype(np.float32)
    # pre-bias the input: xp = x + t (epilogue residual); signs recover
    # sign(x) on-device via ACT bias=-t
    xs = xs + t32[None, None, :, None]
    st = np.stack([inv.astype(np.float32), -t32], axis=1)  # [P, 2] = [s, -t]
    st = np.ascontiguousarray(st)
    return [{"x": xs[i], "w": wT, "st": st} for i in range(_NCORES)]


def _run(inputs, trace=False, trace_cores=None):
    from concourse.bass_utils import run_bass_kernel_spmd

    nc = _get_program()
    in_maps = _prep_inputs(**inputs)
    res = run_bass_kernel_spmd(
        nc,
        in_maps,
        list(range(_NCORES)),
        trace=trace,
        trace_cores=trace_cores,
    )
    out = np.stack(
        [np.asarray(res.results[i]["o"]).astype(np.float32) for i in range(_NCORES)],
        axis=0,
    )
    out = out.reshape(_N, _P, _H, _W)
    return out, res


def kernel(**inputs):
    out, _ = _run(inputs, trace=False)
    return out


# revision 37
# speedup vs baseline: 1.0290x; 1.0290x over previous
"""Binary-conv BasicBlock (sign-act 3x3 binary conv + BN(eval) + residual).

Full shapes: x (32,128,56,56) f32, weight (128,128,3,3), BN params (128,).
Strategy: data-parallel over batch N across 8 NeuronCores (4 images/core).

Per image on-device (fp8 DoubleRow formulation — HW-benched: DR matmuls
stream at 1 column/cycle with 256-deep contraction, s2s = N/2.4GHz + 2.5ns):
  - sign(x) on ScalarE into a zero-framed fp8 tile with 64-wide rows
    (58 rows; cols 57..63 junk-zero).  A second "slot" holds the same
    rows shifted left by 2 cols (VectorE copy), so kh=2's kw=0/kw=2 taps
    pair into one DR matmul via the inter-slot j-step (3712B, %16==0).
  - conv per 7-row chunk = 5 matmuls streaming 448 cols each (full
    64-wide rows; kw shift folded into the rhs offset so all taps land
    on the same psum grid; cols 56..63 of each row-block are junk):
      3x DR (kh0+kh1 pairs @ kw0/1/2, j-step 64B)
      1x DR (kh2: kw0 + kw2-via-slot1, j-step 3712B)
      1x plain (kh2 @ kw1)
    = 2240 streamed cols/chunk vs 3528 for the 9-tap bf16 version.
  - epilogue on VectorE: out = (psum * s) + (x + t) via
    scalar_tensor_tensor, strided psum read (64,7)x(1,56), bf16 out
    (stores halve; bf16 quantization ~0.3% << 2e-2 tolerance).
    The input is pre-biased on the host (xp = x + t) so the residual
    reads the input tile directly; signs recover sign(x) via ACT
    bias=-t.  ScalarE per image = 3 signs + 2 tick separators, under
    the 7.57us PE window.
  - startup: every dma_start costs ~600ns descriptor-gen and the two
    DGE queues share the 16 SDMA engines (~257GB/s sustained total),
    so startup loads all ride the sync queue, w first; steady-state
    x loads go via the scalar queue, stores via sync.  40 warmup
    matmuls bridge preamble-end to first-deps-ready (a PE idle gap
    resets the HAM clock ramp to 1.2GHz for ~3us - measured).
"""

import numpy as np
import ml_dtypes

_N, _C, _H, _W = 32, 128, 56, 56
_P = 128
_NCORES = 8
_NPI = _N // _NCORES  # images per core
_WP = 64              # padded fp8 row width (j-step 64B)
_HP = _H + 2          # 58 rows
_SLOT = _HP * _WP     # 3712 B/partition per slot
_NPIX = _H * _W
_BN_EPS = 1e-5
_CH = 7               # output rows per PSUM bank chunk
_NCH = _H // _CH      # 8 chunks per image
_NPAIR = _NCH // 2    # 4 psum pair-tiles (2 banks each) per image
_CN = _CH * _W        # 392 valid elems per chunk
_CS = _CH * _WP       # 448 streamed columns per chunk

_cache = {}


def _build_program():
    import concourse.bass as bass
    import concourse.bacc as bacc
    import concourse.mybir as mybir
    import concourse.tile as tile

    f32 = mybir.dt.float32
    bf16 = mybir.dt.bfloat16
    fp8 = mybir.dt.float8e4
    DR = mybir.MatmulPerfMode.DoubleRow

    nc = bacc.Bacc("TRN2", target_bir_lowering=False, debug=False)

    # "x" is pre-biased on the host: xp = x + t (t = BN shift, per channel).
    # The epilogue residual needs (x + t) anyway, and the signs recover
    # sign(x) via the ACT bias (-t) — this deletes the whole per-image
    # xp=x+t ScalarE pass (~1.9us/image, Scalar was the bottleneck).
    x_d = nc.dram_tensor("x", [_NPI, _C, _NPIX], f32, kind="ExternalInput")
    w_d = nc.dram_tensor("w", [_C, 9, _P], fp8, kind="ExternalInput")
    st_d = nc.dram_tensor("st", [_P, 2], f32, kind="ExternalInput")
    o_d = nc.dram_tensor("o", [_NPI, _P, _NPIX], bf16, kind="ExternalOutput")

    SIGN = mybir.ActivationFunctionType.Sign
    IDENT = mybir.ActivationFunctionType.Identity
    MULT, ADD = mybir.AluOpType.mult, mybir.AluOpType.add

    with tile.TileContext(nc) as tc:
        with (
            tc.tile_pool(name="const", bufs=1) as cpool,
            tc.tile_pool(name="xin", bufs=4) as xpool,
            tc.tile_pool(name="apad", bufs=1) as apool,
            tc.tile_pool(name="outp", bufs=6) as opool,
            tc.tile_pool(name="ps", bufs=4, space="PSUM") as pspool,
        ):
            # Warmup source on GpSimd (its preamble ends ~1.2us before
            # Vector's, so warmups start right at Tensor preamble end).
            dummy = cpool.tile([_C, _P], fp8)
            nc.gpsimd.memset(dummy[:], 0.0)
            # Throwaway Sign so the 1.3us ACT_TABLE_LOAD runs during the
            # initial DMA wait.
            scratch = cpool.tile([_C, 8], fp8)
            nc.scalar.sign(scratch[:], dummy[:, 0:8])

            x_tiles = [None] * _NPI

            def load_x(n, ranges, eng=None):
                """x loads; images 2-3 go via the Scalar DGE queue so input
                transfers don't serialize behind output stores on Sync (the
                single queue sustains only ~257GB/s, measured)."""
                eng = eng or nc.sync
                if x_tiles[n] is not None:
                    x_t = x_tiles[n]
                else:
                    x_t = xpool.tile([_C, _NPIX], f32, name="x_t", tag="x")
                    x_tiles[n] = x_t
                for r0, r1 in ranges:
                    eng.dma_start(
                        x_t[:, r0 * _W : r1 * _W],
                        x_d[n, :, r0 * _W : r1 * _W],
                    )

            IMG0_RANGES = [(0, 8), (8, 22), (22, 42), (42, 56)]

            # All startup loads on the sync queue, w first (it gates the
            # first real matmul's LDW): the two DGE queues SHARE the 16
            # SDMA engines, so splitting startup transfers across queues
            # just halves each one's rate (measured: +3.4us on mm0).
            wt = cpool.tile([_C, 9, _P], fp8)
            nc.sync.dma_start(wt[:], w_d[:])
            load_x(0, IMG0_RANGES[:1])
            st_t = cpool.tile([_P, 2], f32)
            nc.sync.dma_start(st_t[:], st_d[:])
            load_x(0, IMG0_RANGES[1:])
            s_ap = st_t[:, 0:1]
            nt_ap = st_t[:, 1:2]  # -t: sign(x) = Sign(xp + (-t))

            # Two persistent double-slot fp8 sign tiles (ping-pong across
            # images).  Only the frame/junk cells are zeroed (once):
            #   slot0: top row, bottom row, col 0, cols 57..63
            #   slot1: rows 0..1 (wrap-read slack), row 57, cols 56..63
            a_tiles = []
            for i in range(2):
                a_t = apool.tile([_C, 2 * _SLOT], fp8, name=f"apad{i}", tag=f"apad{i}")
                part = tuple(a_t[:, 0:1].ap[0])
                base = int(a_t[:, 0:1].offset)

                def ap_of(off, dims):
                    return bass.AP(tensor=a_t.tensor, offset=base + off, ap=[part] + dims)

                nc.vector.memset(a_t[:, 0:_WP], 0.0)                      # s0 top
                nc.vector.memset(a_t[:, 57 * _WP : _SLOT], 0.0)           # s0 bottom
                nc.vector.memset(ap_of(0, [(_WP, _HP), (1, 1)]), 0.0)     # s0 col0
                nc.vector.memset(ap_of(57, [(_WP, _HP), (1, 7)]), 0.0)    # s0 c57-63
                nc.vector.memset(a_t[:, _SLOT : _SLOT + 2 * _WP], 0.0)    # s1 r0-1
                nc.vector.memset(a_t[:, _SLOT + 57 * _WP :], 0.0)         # s1 r57
                nc.vector.memset(
                    ap_of(_SLOT + 55, [(_WP, _HP), (1, 9)]), 0.0          # s1 c55-63
                )
                a_tiles.append(a_t)

            def sign_slice(n, r0, r1):
                """ScalarE: sign(x rows r0..r1) -> slot0 rows 1+r0..1+r1."""
                x_v = x_tiles[n][:].rearrange("c (h w) -> c h w", h=_H)
                a_v = a_tiles[n % 2][:, 0:_SLOT].rearrange("c (h w) -> c h w", w=_WP)
                nc.scalar.activation(
                    a_v[:, 1 + r0 : 1 + r1, 1 : _W + 1],
                    x_v[:, r0:r1, :],
                    SIGN,
                    bias=nt_ap,
                )

            def sign2_slice(n, r0, r1):
                """ScalarE: slot1[r, u] = a[r, u+2] = sign(x[r-1, u+1]) — the
                shift-by-2 copy computed directly from x (keeps VectorE free
                for epilogues).  x rows r0..r1 -> slot1 rows 1+r0..1+r1,
                cols 0..54 (col 55 is the a-col-57 frame zero, pre-memset)."""
                x_v = x_tiles[n][:].rearrange("c (h w) -> c h w", h=_H)
                a1_v = a_tiles[n % 2][:, _SLOT:].rearrange("c (h w) -> c h w", w=_WP)
                nc.scalar.activation(
                    a1_v[:, 1 + r0 : 1 + r1, 0:55],
                    x_v[:, r0:r1, 1:56],
                    SIGN,
                    bias=nt_ap,
                )

            # Tiny ScalarE op: dependency "tick" separator.  Dependents of
            # the op before it release ~60ns after it instead of waiting
            # through the next full-size ACT (the sem-tick granularity
            # otherwise rounds the release up to the next op's end).
            tick_t = cpool.tile([_C, 8], bf16)

            def tick():
                nc.scalar.activation(tick_t[:], dummy[:, 0:8], IDENT)

            # Image-0 staging: signs only (residual uses the pre-biased
            # input tile directly; image 0 uses the 6-mm no-slot1 form).
            sign_slice(0, *IMG0_RANGES[0])
            tick()
            sign_slice(0, *IMG0_RANGES[1])
            tick()
            sign_slice(0, *IMG0_RANGES[2])
            tick()
            sign_slice(0, *IMG0_RANGES[3])
            tick()

            def stage_sign(n, ranges):
                """ScalarE staging for images 1..3.  sign2 only for the first
                half: the second half's slot1 rows come from a VectorE copy
                (stage_copy) so Scalar stays under the per-image budget."""
                (a0, a1), (b0, b1) = ranges
                sign_slice(n, a0, a1)
                sign2_slice(n, max(a0, 1), a1)
                tick()
                sign_slice(n, b0, b1)
                tick()

            def stage_copy(n):
                """VectorE: slot1 rows 29..57 = slot0 rows 29..57 shifted by
                2 cols (row 57 reads the frame zeros).  Emitted at the END of
                the previous image's pair loop so it can't head-block that
                image's epilogue stts."""
                a_t = a_tiles[n % 2]
                av = a_t[:].rearrange("c (s h w) -> c s h w", s=2, w=_WP)
                nc.vector.tensor_copy(
                    av[:, 1, 29:58, 0:56],
                    av[:, 0, 29:58, 2:58],
                )

            # PE warmup (results discarded).  ~107ns apart at mid-clock;
            # covers preamble-end -> first-deps-ready (~10.8us: sign0 after
            # the startup DGE chain).  A PE idle gap here resets the HAM
            # clock ramp, so over-covering beats under-covering.
            warm_ps = pspool.tile([_P, 2, 512], f32, name="warm_ps", tag="ps")
            for i in range(40):
                nc.tensor.matmul(
                    warm_ps[:, i % 2, :128],
                    dummy[:],
                    dummy[:],
                    start=True,
                    stop=True,
                )

            def chunk_mms(n, bank_ap, rb, use_slot1):
                """Matmuls for the chunk at a-row rb (out rows rb..rb+6),
                bank-major (tap-major pairs were tried: steady state gained
                only ~35ns/image but image-0's pair-0 then gates on sign1 —
                net loss).
                use_slot1: 5-mm form (needs slot1 copies done); else 6-mm
                form with kh=2 as three plain taps (image 0: avoids coupling
                the startup-critical first chunks to the copy pipeline)."""
                a_t = a_tiles[n % 2]
                part = tuple(a_t[:, 0:1].ap[0])
                base = int(a_t[:, 0:1].offset)

                def rhs(off, step):
                    dims = [part]
                    if step is not None:
                        dims.append((step, 2))
                    dims.append((1, _CS))
                    return bass.AP(tensor=a_t.tensor, offset=base + off, ap=dims)

                plan = [
                    (wt[:, 0:2, :], rb * _WP + 0, _WP, DR),
                    (wt[:, 2:4, :], rb * _WP + 1, _WP, DR),
                    (wt[:, 4:6, :], rb * _WP + 2, _WP, DR),
                ]
                if use_slot1:
                    plan += [
                        (wt[:, 6:8, :], (rb + 2) * _WP + 0, _SLOT, DR),
                        (wt[:, 8, :], (rb + 2) * _WP + 1, None, None),
                    ]
                else:
                    plan += [
                        (wt[:, 6, :], (rb + 2) * _WP + 0, None, None),
                        (wt[:, 8, :], (rb + 2) * _WP + 1, None, None),
                        (wt[:, 7, :], (rb + 2) * _WP + 2, None, None),
                    ]
                for i, (w_ap, off, step, pm) in enumerate(plan):
                    nc.tensor.matmul(
                        bank_ap,
                        w_ap,
                        rhs(off, step),
                        start=(i == 0),
                        stop=(i == len(plan) - 1),
                        perf_mode=pm,
                    )

            for n in range(_NPI):
                if n + 1 < _NPI:
                    # image 1's loads on sync (startup transfers still
                    # draining there; a second queue would steal SDMA
                    # engines); images 2-3 via the scalar queue.
                    load_x(n + 1, [(0, 28), (28, 56)],
                           eng=(nc.sync if n == 0 else nc.scalar))
                    stage_sign(n + 1, [(0, 28), (28, 56)])

                last_img = n == _NPI - 1

                for p in range(_NPAIR):
                    fine_tail = last_img and p == _NPAIR - 1
                    if fine_tail:
                        bank_tiles = [
                            pspool.tile([_P, 512], f32, name=f"pstb{b}", tag="ps")
                            for b in range(2)
                        ]
                        bank_aps = [bt[:, :_CS] for bt in bank_tiles]
                        bank_views = [
                            bt[:].rearrange("c (h w) -> c h w", w=_WP)
                            for bt in bank_tiles
                        ]
                    else:
                        pst = pspool.tile([_P, 2, 512], f32, name="pst", tag="ps")
                        bank_aps = [pst[:, b, :_CS] for b in range(2)]
                        pv = pst[:].rearrange("c b (h w) -> c b h w", w=_WP)
                        bank_views = [pv[:, b] for b in range(2)]
                    out_t = opool.tile([_P, 2 * _CN], bf16, name="out_t", tag="o")

                    def epi(b, h0, h1):
                        """stt halves/full of bank b into out_t: out =
                        psum*s + (x+t), the residual read straight from the
                        pre-biased input tile."""
                        nc.vector.scalar_tensor_tensor(
                            out_t[:, b * _CN + h0 * _W : b * _CN + h1 * _W],
                            bank_views[b][:, h0:h1, 0:56],
                            s_ap,
                            x_tiles[n][
                                :, (2 * p + b) * _CN + h0 * _W :
                            ][:, : (h1 - h0) * _W],
                            MULT,
                            ADD,
                        )

                    for b in range(2):
                        chunk_mms(n, bank_aps[b], (2 * p + b) * _CH,
                                  use_slot1=(n > 0))
                        if fine_tail:
                            c0 = (2 * p + b) * _CN
                            if b == 0:
                                # bank 0's store DGE rides the (idle) scalar
                                # queue so the three tail store issues don't
                                # serialize ~1.8us on sync
                                epi(b, 0, _CH)
                                nc.scalar.dma_start(
                                    o_d[n, :, c0 : c0 + _CN],
                                    out_t[:, 0:_CN],
                                )
                            else:
                                for h0, h1 in ((0, 4), (4, _CH)):
                                    epi(b, h0, h1)
                                    nc.sync.dma_start(
                                        o_d[n, :, c0 + h0 * _W : c0 + h1 * _W],
                                        out_t[
                                            :, _CN + h0 * _W : _CN + h1 * _W
                                        ],
                                    )
                    if not fine_tail:
                        # per-bank stts (walrus limits STT inputs to 2D/3D
                        # APs, so the two banks can't merge into one 4D op)
                        for b in range(2):
                            epi(b, 0, _CH)
                        nc.sync.dma_start(
                            o_d[n, :, p * 2 * _CN : (p + 1) * 2 * _CN],
                            out_t[:],
                        )
                        if p == _NPAIR - 1 and n + 1 < _NPI:
                            stage_copy(n + 1)

    nc.compile()
    return nc


def _get_program():
    if "nc" not in _cache:
        _cache["nc"] = _build_program()
    return _cache["nc"]


def _prep_inputs(x, weight, bias, gamma, beta, running_mean, running_var):
    # per-core batch shards
    xs = np.ascontiguousarray(
        np.asarray(x, dtype=np.float32).reshape(_NCORES, _NPI, _C, _NPIX)
    )
    # sign(weight) packed as [C, k, P] fp8, k-order =
    # [(0,0),(1,0),(0,1),(1,1),(0,2),(1,2),(2,0),(2,2),(2,1)] (kh,kw)
    wb = np.sign(np.asarray(weight, dtype=np.float32))  # [P, C, 3, 3]
    korder = [(0, 0), (1, 0), (0, 1), (1, 1), (0, 2), (1, 2), (2, 0), (2, 2), (2, 1)]
    wT = np.stack(
        [wb[:, :, kh, kw].T for kh, kw in korder], axis=1
    )  # [C, 9, P]
    wT = np.ascontiguousarray(wT).astype(ml_dtypes.float8_e4m3)
    inv = np.asarray(gamma, dtype=np.float64) / np.sqrt(
        np.asarray(running_var, dtype=np.float64) + _BN_EPS
    )
    shift = (
        np.asarray(bias, dtype=np.float64) * inv
        + np.asarray(beta, dtype=np.float64)
        - np.asarray(running_mean, dtype=np.float64) * inv
    )
    t32 = shift.astype(np.float32)
    # pre-bias the input: xp = x + t (epilogue residual); signs recover
    # sign(x) on-device via ACT bias=-t
    xs = xs + t32[None, None, :, None]
    st = np.stack([inv.astype(np.float32), -t32], axis=1)  # [P, 2] = [s, -t]
    st = np.ascontiguousarray(st)
    return [{"x": xs[i], "w": wT, "st": st} for i in range(_NCORES)]


def _run(inputs, trace=False, trace_cores=None):
    from concourse.bass_utils import run_bass_kernel_spmd

    nc = _get_program()
    in_maps = _prep_inputs(**inputs)
    res = run_bass_kernel_spmd(
        nc,
        in_maps,
        list(range(_NCORES)),
        trace=trace,
        trace_cores=trace_cores,
    )
    out = np.stack(
        [np.asarray(res.results[i]["o"]).astype(np.float32) for i in range(_NCORES)],
        axis=0,
    )
    out = out.reshape(_N, _P, _H, _W)
    return out, res


def kernel(**inputs):
    out, _ = _run(inputs, trace=False)
    return out


# revision 40
# speedup vs baseline: 1.0365x; 1.0072x over previous
"""Binary-conv BasicBlock (sign-act 3x3 binary conv + BN(eval) + residual).

Full shapes: x (32,128,56,56) f32, weight (128,128,3,3), BN params (128,).
Strategy: data-parallel over batch N across 8 NeuronCores (4 images/core).

Per image on-device (fp8 DoubleRow formulation — HW-benched: DR matmuls
stream at 1 column/cycle with 256-deep contraction, s2s = N/2.4GHz + 2.5ns):
  - sign(x) on ScalarE into a zero-framed fp8 tile with 64-wide rows
    (58 rows; cols 57..63 junk-zero).  A second "slot" holds the same
    rows shifted left by 2 cols (VectorE copy), so kh=2's kw=0/kw=2 taps
    pair into one DR matmul via the inter-slot j-step (3712B, %16==0).
  - conv per 7-row chunk = 5 matmuls streaming 448 cols each (full
    64-wide rows; kw shift folded into the rhs offset so all taps land
    on the same psum grid; cols 56..63 of each row-block are junk):
      3x DR (kh0+kh1 pairs @ kw0/1/2, j-step 64B)
      1x DR (kh2: kw0 + kw2-via-slot1, j-step 3712B)
      1x plain (kh2 @ kw1)
    = 2240 streamed cols/chunk vs 3528 for the 9-tap bf16 version.
  - epilogue on VectorE: out = (psum * s) + (x + t) via
    scalar_tensor_tensor, strided psum read (64,7)x(1,56), bf16 out
    (stores halve; bf16 quantization ~0.3% << 2e-2 tolerance).
    The input is pre-biased on the host (xp = x + t) so the residual
    reads the input tile directly; signs recover sign(x) via ACT
    bias=-t.  ScalarE per image = 3 signs + 2 tick separators, under
    the 7.57us PE window.
  - startup: every dma_start costs ~600ns descriptor-gen and the two
    DGE queues share the 16 SDMA engines (~257GB/s sustained total),
    so startup loads all ride the sync queue, w first; steady-state
    x loads go via the scalar queue, stores via sync.  40 warmup
    matmuls bridge preamble-end to first-deps-ready (a PE idle gap
    resets the HAM clock ramp to 1.2GHz for ~3us - measured).
"""

import numpy as np
import ml_dtypes

_N, _C, _H, _W = 32, 128, 56, 56
_P = 128
_NCORES = 8
_NPI = _N // _NCORES  # images per core
_WP = 64              # padded fp8 row width (j-step 64B)
_HP = _H + 2          # 58 rows
_SLOT = _HP * _WP     # 3712 B/partition per slot
_NPIX = _H * _W
_BN_EPS = 1e-5
_CH = 7               # output rows per PSUM bank chunk
_NCH = _H // _CH      # 8 chunks per image
_NPAIR = _NCH // 2    # 4 psum pair-tiles (2 banks each) per image
_CN = _CH * _W        # 392 valid elems per chunk
_CS = _CH * _WP       # 448 streamed columns per chunk

_cache = {}


def _build_program():
    import concourse.bass as bass
    import concourse.bacc as bacc
    import concourse.mybir as mybir
    import concourse.tile as tile

    f32 = mybir.dt.float32
    bf16 = mybir.dt.bfloat16
    fp8 = mybir.dt.float8e4
    DR = mybir.MatmulPerfMode.DoubleRow

    nc = bacc.Bacc("TRN2", target_bir_lowering=False, debug=False)

    # "x" is pre-biased on the host: xp = x + t (t = BN shift, per channel).
    # The epilogue residual needs (x + t) anyway, and the signs recover
    # sign(x) via the ACT bias (-t) — this deletes the whole per-image
    # xp=x+t ScalarE pass (~1.9us/image, Scalar was the bottleneck).
    x_d = nc.dram_tensor("x", [_NPI, _C, _NPIX], f32, kind="ExternalInput")
    w_d = nc.dram_tensor("w", [_C, 9, _P], fp8, kind="ExternalInput")
    st_d = nc.dram_tensor("st", [_P, 2], f32, kind="ExternalInput")
    o_d = nc.dram_tensor("o", [_NPI, _P, _NPIX], bf16, kind="ExternalOutput")

    SIGN = mybir.ActivationFunctionType.Sign
    IDENT = mybir.ActivationFunctionType.Identity
    MULT, ADD = mybir.AluOpType.mult, mybir.AluOpType.add

    with tile.TileContext(nc) as tc:
        with (
            tc.tile_pool(name="const", bufs=1) as cpool,
            tc.tile_pool(name="xin", bufs=4) as xpool,
            tc.tile_pool(name="apad", bufs=1) as apool,
            tc.tile_pool(name="outp", bufs=6) as opool,
            tc.tile_pool(name="ps", bufs=4, space="PSUM") as pspool,
        ):
            # Warmup source on GpSimd (its preamble ends ~1.2us before
            # Vector's, so warmups start right at Tensor preamble end).
            dummy = cpool.tile([_C, _P], fp8)
            nc.gpsimd.memset(dummy[:], 0.0)
            # Throwaway Sign so the 1.3us ACT_TABLE_LOAD runs during the
            # initial DMA wait.
            scratch = cpool.tile([_C, 8], fp8)
            nc.scalar.sign(scratch[:], dummy[:, 0:8])

            x_tiles = [None] * _NPI

            def load_x(n, ranges, eng=None):
                """x loads; images 2-3 go via the Scalar DGE queue so input
                transfers don't serialize behind output stores on Sync (the
                single queue sustains only ~257GB/s, measured)."""
                eng = eng or nc.sync
                if x_tiles[n] is not None:
                    x_t = x_tiles[n]
                else:
                    x_t = xpool.tile([_C, _NPIX], f32, name="x_t", tag="x")
                    x_tiles[n] = x_t
                for r0, r1 in ranges:
                    eng.dma_start(
                        x_t[:, r0 * _W : r1 * _W],
                        x_d[n, :, r0 * _W : r1 * _W],
                    )

            IMG0_RANGES = [(0, 8), (8, 22), (22, 42), (42, 56)]

            # All startup loads on the sync queue: the two DGE queues SHARE
            # the 16 SDMA engines, so splitting startup transfers across
            # queues just halves each one's rate (measured: +3.4us on mm0).
            # x0 rows 0-8 ride first: sign0's chain (transfer -> sign ->
            # sem) is longer than w's (transfer -> LDW pull-ahead).
            load_x(0, IMG0_RANGES[:1])
            wt = cpool.tile([_C, 9, _P], fp8)
            nc.sync.dma_start(wt[:], w_d[:])
            st_t = cpool.tile([_P, 2], f32)
            nc.sync.dma_start(st_t[:], st_d[:])
            load_x(0, IMG0_RANGES[1:])
            s_ap = st_t[:, 0:1]
            nt_ap = st_t[:, 1:2]  # -t: sign(x) = Sign(xp + (-t))

            # Two persistent double-slot fp8 sign tiles (ping-pong across
            # images).  Only the frame/junk cells are zeroed (once):
            #   slot0: top row, bottom row, col 0, cols 57..63
            #   slot1: rows 0..1 (wrap-read slack), row 57, cols 56..63
            a_tiles = []
            for i in range(2):
                a_t = apool.tile([_C, 2 * _SLOT], fp8, name=f"apad{i}", tag=f"apad{i}")
                part = tuple(a_t[:, 0:1].ap[0])
                base = int(a_t[:, 0:1].offset)

                def ap_of(off, dims):
                    return bass.AP(tensor=a_t.tensor, offset=base + off, ap=[part] + dims)

                nc.vector.memset(a_t[:, 0:_WP], 0.0)                      # s0 top
                nc.vector.memset(a_t[:, 57 * _WP : _SLOT], 0.0)           # s0 bottom
                nc.vector.memset(ap_of(0, [(_WP, _HP), (1, 1)]), 0.0)     # s0 col0
                nc.vector.memset(ap_of(57, [(_WP, _HP), (1, 7)]), 0.0)    # s0 c57-63
                nc.vector.memset(a_t[:, _SLOT : _SLOT + 2 * _WP], 0.0)    # s1 r0-1
                nc.vector.memset(a_t[:, _SLOT + 57 * _WP :], 0.0)         # s1 r57
                nc.vector.memset(
                    ap_of(_SLOT + 55, [(_WP, _HP), (1, 9)]), 0.0          # s1 c55-63
                )
                a_tiles.append(a_t)

            def sign_slice(n, r0, r1):
                """ScalarE: sign(x rows r0..r1) -> slot0 rows 1+r0..1+r1."""
                x_v = x_tiles[n][:].rearrange("c (h w) -> c h w", h=_H)
                a_v = a_tiles[n % 2][:, 0:_SLOT].rearrange("c (h w) -> c h w", w=_WP)
                nc.scalar.activation(
                    a_v[:, 1 + r0 : 1 + r1, 1 : _W + 1],
                    x_v[:, r0:r1, :],
                    SIGN,
                    bias=nt_ap,
                )

            def sign2_slice(n, r0, r1):
                """ScalarE: slot1[r, u] = a[r, u+2] = sign(x[r-1, u+1]) — the
                shift-by-2 copy computed directly from x (keeps VectorE free
                for epilogues).  x rows r0..r1 -> slot1 rows 1+r0..1+r1,
                cols 0..54 (col 55 is the a-col-57 frame zero, pre-memset)."""
                x_v = x_tiles[n][:].rearrange("c (h w) -> c h w", h=_H)
                a1_v = a_tiles[n % 2][:, _SLOT:].rearrange("c (h w) -> c h w", w=_WP)
                nc.scalar.activation(
                    a1_v[:, 1 + r0 : 1 + r1, 0:55],
                    x_v[:, r0:r1, 1:56],
                    SIGN,
                    bias=nt_ap,
                )

            # Tiny ScalarE op: dependency "tick" separator.  Dependents of
            # the op before it release ~60ns after it instead of waiting
            # through the next full-size ACT (the sem-tick granularity
            # otherwise rounds the release up to the next op's end).
            tick_t = cpool.tile([_C, 8], bf16)

            def tick():
                nc.scalar.activation(tick_t[:], dummy[:, 0:8], IDENT)

            # Image-0 staging: signs only (residual uses the pre-biased
            # input tile directly; image 0 uses the 6-mm no-slot1 form).
            sign_slice(0, *IMG0_RANGES[0])
            tick()
            sign_slice(0, *IMG0_RANGES[1])
            tick()
            sign_slice(0, *IMG0_RANGES[2])
            tick()
            sign_slice(0, *IMG0_RANGES[3])
            tick()

            def stage_sign(n, ranges):
                """ScalarE staging for images 1..3.  sign2 only for the first
                half: the second half's slot1 rows come from a VectorE copy
                (stage_copy) so Scalar stays under the per-image budget."""
                (a0, a1), (b0, b1) = ranges
                sign_slice(n, a0, a1)
                sign2_slice(n, max(a0, 1), a1)
                tick()
                sign_slice(n, b0, b1)
                tick()

            def stage_copy(n):
                """VectorE: slot1 rows 29..57 = slot0 rows 29..57 shifted by
                2 cols (row 57 reads the frame zeros).  Emitted at the END of
                the previous image's pair loop so it can't head-block that
                image's epilogue stts."""
                a_t = a_tiles[n % 2]
                av = a_t[:].rearrange("c (s h w) -> c s h w", s=2, w=_WP)
                nc.vector.tensor_copy(
                    av[:, 1, 29:58, 0:56],
                    av[:, 0, 29:58, 2:58],
                )

            # PE warmup (results discarded).  ~107ns apart at mid-clock;
            # covers preamble-end -> first-deps-ready (~10.8us: sign0 after
            # the startup DGE chain).  A PE idle gap here resets the HAM
            # clock ramp, so over-covering beats under-covering.
            warm_ps = pspool.tile([_P, 2, 512], f32, name="warm_ps", tag="ps")
            for i in range(48):
                nc.tensor.matmul(
                    warm_ps[:, i % 2, :128],
                    dummy[:],
                    dummy[:],
                    start=True,
                    stop=True,
                )

            def chunk_mms(n, bank_ap, rb, use_slot1):
                """Matmuls for the chunk at a-row rb (out rows rb..rb+6),
                bank-major (tap-major pairs were tried: steady state gained
                only ~35ns/image but image-0's pair-0 then gates on sign1 —
                net loss).
                use_slot1: 5-mm form (needs slot1 copies done); else 6-mm
                form with kh=2 as three plain taps (image 0: avoids coupling
                the startup-critical first chunks to the copy pipeline)."""
                a_t = a_tiles[n % 2]
                part = tuple(a_t[:, 0:1].ap[0])
                base = int(a_t[:, 0:1].offset)

                def rhs(off, step):
                    dims = [part]
                    if step is not None:
                        dims.append((step, 2))
                    dims.append((1, _CS))
                    return bass.AP(tensor=a_t.tensor, offset=base + off, ap=dims)

                plan = [
                    (wt[:, 0:2, :], rb * _WP + 0, _WP, DR),
                    (wt[:, 2:4, :], rb * _WP + 1, _WP, DR),
                    (wt[:, 4:6, :], rb * _WP + 2, _WP, DR),
                ]
                if use_slot1:
                    plan += [
                        (wt[:, 6:8, :], (rb + 2) * _WP + 0, _SLOT, DR),
                        (wt[:, 8, :], (rb + 2) * _WP + 1, None, None),
                    ]
                else:
                    plan += [
                        (wt[:, 6, :], (rb + 2) * _WP + 0, None, None),
                        (wt[:, 8, :], (rb + 2) * _WP + 1, None, None),
                        (wt[:, 7, :], (rb + 2) * _WP + 2, None, None),
                    ]
                for i, (w_ap, off, step, pm) in enumerate(plan):
                    nc.tensor.matmul(
                        bank_ap,
                        w_ap,
                        rhs(off, step),
                        start=(i == 0),
                        stop=(i == len(plan) - 1),
                        perf_mode=pm,
                    )

            for n in range(_NPI):
                if n + 1 < _NPI:
                    # image 1's loads on sync (startup transfers still
                    # draining there; a second queue would steal SDMA
                    # engines); images 2-3 via the scalar queue.
                    load_x(n + 1, [(0, 28), (28, 56)],
                           eng=(nc.sync if n == 0 else nc.scalar))
                    stage_sign(n + 1, [(0, 28), (28, 56)])

                last_img = n == _NPI - 1

                for p in range(_NPAIR):
                    fine_tail = last_img and p == _NPAIR - 1
                    if fine_tail:
                        bank_tiles = [
                            pspool.tile([_P, 512], f32, name=f"pstb{b}", tag="ps")
                            for b in range(2)
                        ]
                        bank_aps = [bt[:, :_CS] for bt in bank_tiles]
                        bank_views = [
                            bt[:].rearrange("c (h w) -> c h w", w=_WP)
                            for bt in bank_tiles
                        ]
                    else:
                        pst = pspool.tile([_P, 2, 512], f32, name="pst", tag="ps")
                        bank_aps = [pst[:, b, :_CS] for b in range(2)]
                        pv = pst[:].rearrange("c b (h w) -> c b h w", w=_WP)
                        bank_views = [pv[:, b] for b in range(2)]
                    out_t = opool.tile([_P, 2 * _CN], bf16, name="out_t", tag="o")

                    def epi(b, h0, h1):
                        """stt halves/full of bank b into out_t: out =
                        psum*s + (x+t), the residual read straight from the
                        pre-biased input tile."""
                        nc.vector.scalar_tensor_tensor(
                            out_t[:, b * _CN + h0 * _W : b * _CN + h1 * _W],
                            bank_views[b][:, h0:h1, 0:56],
                            s_ap,
                            x_tiles[n][
                                :, (2 * p + b) * _CN + h0 * _W :
                            ][:, : (h1 - h0) * _W],
                            MULT,
                            ADD,
                        )

                    for b in range(2):
                        chunk_mms(n, bank_aps[b], (2 * p + b) * _CH,
                                  use_slot1=(n > 0))
                        if fine_tail:
                            c0 = (2 * p + b) * _CN
                            if b == 0:
                                # bank 0's store DGE rides the (idle) scalar
                                # queue so the three tail store issues don't
                                # serialize ~1.8us on sync
                                epi(b, 0, _CH)
                                nc.scalar.dma_start(
                                    o_d[n, :, c0 : c0 + _CN],
                                    out_t[:, 0:_CN],
                                )
                            else:
                                # halves on alternating queues: the two
                                # ~600ns DGE issues overlap instead of
                                # serializing after the final stt
                                for (h0, h1), eng in (
                                    ((0, 4), nc.scalar),
                                    ((4, _CH), nc.sync),
                                ):
                                    epi(b, h0, h1)
                                    eng.dma_start(
                                        o_d[n, :, c0 + h0 * _W : c0 + h1 * _W],
                                        out_t[
                                            :, _CN + h0 * _W : _CN + h1 * _W
                                        ],
                                    )
                    if not fine_tail:
                        # per-bank stts (walrus limits STT inputs to 2D/3D
                        # APs, so the two banks can't merge into one 4D op)
                        for b in range(2):
                            epi(b, 0, _CH)
                        nc.sync.dma_start(
                            o_d[n, :, p * 2 * _CN : (p + 1) * 2 * _CN],
                            out_t[:],
                        )
                        if p == _NPAIR - 1 and n + 1 < _NPI:
                            stage_copy(n + 1)

    nc.compile()
    return nc


def _get_program():
    if "nc" not in _cache:
        _cache["nc"] = _build_program()
    return _cache["nc"]


def _prep_inputs(x, weight, bias, gamma, beta, running_mean, running_var):
    # per-core batch shards
    xs = np.ascontiguousarray(
        np.asarray(x, dtype=np.float32).reshape(_NCORES, _NPI, _C, _NPIX)
    )
    # sign(weight) packed as [C, k, P] fp8, k-order =
    # [(0,0),(1,0),(0,1),(1,1),(0,2),(1,2),(2,0),(2,2),(2,1)] (kh,kw)
    wb = np.sign(np.asarray(weight, dtype=np.float32))  # [P, C, 3, 3]
    korder = [(0, 0), (1, 0), (0, 1), (1, 1), (0, 2), (1, 2), (2, 0), (2, 2), (2, 1)]
    wT = np.stack(
        [wb[:, :, kh, kw].T for kh, kw in korder], axis=1
    )  # [C, 9, P]
    wT = np.ascontiguousarray(wT).astype(ml_dtypes.float8_e4m3)
    inv = np.asarray(gamma, dtype=np.float64) / np.sqrt(
        np.asarray(running_var, dtype=np.float64) + _BN_EPS
    )
    shift = (
        np.asarray(bias, dtype=np.float64) * inv
        + np.asarray(beta, dtype=np.float64)
        - np.asarray(running_mean, dtype=np.float64) * inv
    )
    t32 = shift.astype(np.float32)
    # pre-bias the input: xp = x + t (epilogue residual); signs recover
    # sign(x) on-device via ACT bias=-t
    xs = xs + t32[None, None, :, None]
    st = np.stack([inv.astype(np.float32), -t32], axis=1)  # [P, 2] = [s, -t]
    st = np.ascontiguousarray(st)
    return [{"x": xs[i], "w": wT, "st": st} for i in range(_NCORES)]


def _run(inputs, trace=False, trace_cores=None):
    from concourse.bass_utils import run_bass_kernel_spmd

    nc = _get_program()
    in_maps = _prep_inputs(**inputs)
    res = run_bass_kernel_spmd(
        nc,
        in_maps,
        list(range(_NCORES)),
        trace=trace,
        trace_cores=trace_cores,
    )
    out = np.stack(
        [np.asarray(res.results[i]["o"]).astype(np.float32) for i in range(_NCORES)],
        axis=0,
    )
    out = out.reshape(_N, _P, _H, _W)
    return out, res


def kernel(**inputs):
    out, _ = _run(inputs, trace=False)
    return out


# revision 42
# speedup vs baseline: 1.0412x; 1.0046x over previous
"""Binary-conv BasicBlock (sign-act 3x3 binary conv + BN(eval) + residual).

Full shapes: x (32,128,56,56) f32, weight (128,128,3,3), BN params (128,).
Strategy: data-parallel over batch N across 8 NeuronCores (4 images/core).

Per image on-device (fp8 DoubleRow formulation — HW-benched: DR matmuls
stream at 1 column/cycle with 256-deep contraction, s2s = N/2.4GHz + 2.5ns):
  - sign(x) on ScalarE into a zero-framed fp8 tile with 64-wide rows
    (58 rows; cols 57..63 junk-zero).  A second "slot" holds the same
    rows shifted left by 2 cols (VectorE copy), so kh=2's kw=0/kw=2 taps
    pair into one DR matmul via the inter-slot j-step (3712B, %16==0).
  - conv per 7-row chunk = 5 matmuls streaming 448 cols each (full
    64-wide rows; kw shift folded into the rhs offset so all taps land
    on the same psum grid; cols 56..63 of each row-block are junk):
      3x DR (kh0+kh1 pairs @ kw0/1/2, j-step 64B)
      1x DR (kh2: kw0 + kw2-via-slot1, j-step 3712B)
      1x plain (kh2 @ kw1)
    = 2240 streamed cols/chunk vs 3528 for the 9-tap bf16 version.
  - epilogue on VectorE: out = (psum * s) + (x + t) via
    scalar_tensor_tensor, strided psum read (64,7)x(1,56), bf16 out
    (stores halve; bf16 quantization ~0.3% << 2e-2 tolerance).
    The input is pre-biased on the host (xp = x + t) so the residual
    reads the input tile directly; signs recover sign(x) via ACT
    bias=-t.  ScalarE per image = 3 signs + 2 tick separators, under
    the 7.57us PE window.
  - startup: every dma_start costs ~600ns descriptor-gen and the two
    DGE queues share the 16 SDMA engines (~257GB/s sustained total),
    so startup loads all ride the sync queue, w first; steady-state
    x loads go via the scalar queue, stores via sync.  40 warmup
    matmuls bridge preamble-end to first-deps-ready (a PE idle gap
    resets the HAM clock ramp to 1.2GHz for ~3us - measured).
"""

import numpy as np
import ml_dtypes

_N, _C, _H, _W = 32, 128, 56, 56
_P = 128
_NCORES = 8
_NPI = _N // _NCORES  # images per core
_WP = 64              # padded fp8 row width (j-step 64B)
_HP = _H + 2          # 58 rows
_SLOT = _HP * _WP     # 3712 B/partition per slot
_NPIX = _H * _W
_BN_EPS = 1e-5
_CH = 7               # output rows per PSUM bank chunk
_NCH = _H // _CH      # 8 chunks per image
_NPAIR = _NCH // 2    # 4 psum pair-tiles (2 banks each) per image
_CN = _CH * _W        # 392 valid elems per chunk
_CS = _CH * _WP       # 448 streamed columns per chunk

_cache = {}


def _build_program():
    import concourse.bass as bass
    import concourse.bacc as bacc
    import concourse.mybir as mybir
    import concourse.tile as tile

    f32 = mybir.dt.float32
    bf16 = mybir.dt.bfloat16
    fp8 = mybir.dt.float8e4
    DR = mybir.MatmulPerfMode.DoubleRow

    nc = bacc.Bacc("TRN2", target_bir_lowering=False, debug=False)

    # "x" is pre-biased on the host: xp = x + t (t = BN shift, per channel).
    # The epilogue residual needs (x + t) anyway, and the signs recover
    # sign(x) via the ACT bias (-t) — this deletes the whole per-image
    # xp=x+t ScalarE pass (~1.9us/image, Scalar was the bottleneck).
    x_d = nc.dram_tensor("x", [_NPI, _C, _NPIX], f32, kind="ExternalInput")
    w_d = nc.dram_tensor("w", [_C, 9, _P], fp8, kind="ExternalInput")
    st_d = nc.dram_tensor("st", [_P, 2], f32, kind="ExternalInput")
    o_d = nc.dram_tensor("o", [_NPI, _P, _NPIX], bf16, kind="ExternalOutput")

    SIGN = mybir.ActivationFunctionType.Sign
    IDENT = mybir.ActivationFunctionType.Identity
    MULT, ADD = mybir.AluOpType.mult, mybir.AluOpType.add

    with tile.TileContext(nc) as tc:
        with (
            tc.tile_pool(name="const", bufs=1) as cpool,
            tc.tile_pool(name="xin", bufs=4) as xpool,
            tc.tile_pool(name="apad", bufs=1) as apool,
            tc.tile_pool(name="outp", bufs=6) as opool,
            tc.tile_pool(name="ps", bufs=4, space="PSUM") as pspool,
        ):
            # Warmup source on GpSimd (its preamble ends ~1.2us before
            # Vector's, so warmups start right at Tensor preamble end).
            dummy = cpool.tile([_C, _P], fp8)
            nc.gpsimd.memset(dummy[:], 0.0)
            # Throwaway Sign so the 1.3us ACT_TABLE_LOAD runs during the
            # initial DMA wait.
            scratch = cpool.tile([_C, 8], fp8)
            nc.scalar.sign(scratch[:], dummy[:, 0:8])

            x_tiles = [None] * _NPI

            def load_x(n, ranges, eng=None):
                """x loads; images 2-3 go via the Scalar DGE queue so input
                transfers don't serialize behind output stores on Sync (the
                single queue sustains only ~257GB/s, measured)."""
                eng = eng or nc.sync
                if x_tiles[n] is not None:
                    x_t = x_tiles[n]
                else:
                    x_t = xpool.tile([_C, _NPIX], f32, name="x_t", tag="x")
                    x_tiles[n] = x_t
                for r0, r1 in ranges:
                    eng.dma_start(
                        x_t[:, r0 * _W : r1 * _W],
                        x_d[n, :, r0 * _W : r1 * _W],
                    )

            IMG0_RANGES = [(0, 8), (8, 22), (22, 30), (30, 42), (42, 56)]

            # All startup loads on the sync queue: the two DGE queues SHARE
            # the 16 SDMA engines, so splitting startup transfers across
            # queues just halves each one's rate (measured: +3.4us on mm0).
            # x0 rows 0-8 ride first: sign0's chain (transfer -> sign ->
            # sem) is longer than w's (transfer -> LDW pull-ahead).
            load_x(0, IMG0_RANGES[:1])
            wt = cpool.tile([_C, 9, _P], fp8)
            nc.sync.dma_start(wt[:], w_d[:])
            st_t = cpool.tile([_P, 2], f32)
            nc.sync.dma_start(st_t[:], st_d[:])
            load_x(0, IMG0_RANGES[1:])
            s_ap = st_t[:, 0:1]
            nt_ap = st_t[:, 1:2]  # -t: sign(x) = Sign(xp + (-t))

            # Two persistent double-slot fp8 sign tiles (ping-pong across
            # images).  Only the frame/junk cells are zeroed (once):
            #   slot0: top row, bottom row, col 0, cols 57..63
            #   slot1: rows 0..1 (wrap-read slack), row 57, cols 56..63
            a_tiles = []
            for i in range(2):
                a_t = apool.tile([_C, 2 * _SLOT], fp8, name=f"apad{i}", tag=f"apad{i}")
                part = tuple(a_t[:, 0:1].ap[0])
                base = int(a_t[:, 0:1].offset)

                def ap_of(off, dims):
                    return bass.AP(tensor=a_t.tensor, offset=base + off, ap=[part] + dims)

                nc.vector.memset(a_t[:, 0:_WP], 0.0)                      # s0 top
                nc.vector.memset(a_t[:, 57 * _WP : _SLOT], 0.0)           # s0 bottom
                nc.vector.memset(ap_of(0, [(_WP, _HP), (1, 1)]), 0.0)     # s0 col0
                nc.vector.memset(ap_of(57, [(_WP, _HP), (1, 7)]), 0.0)    # s0 c57-63
                nc.vector.memset(a_t[:, _SLOT : _SLOT + 2 * _WP], 0.0)    # s1 r0-1
                nc.vector.memset(a_t[:, _SLOT + 57 * _WP :], 0.0)         # s1 r57
                nc.vector.memset(
                    ap_of(_SLOT + 55, [(_WP, _HP), (1, 9)]), 0.0          # s1 c55-63
                )
                a_tiles.append(a_t)

            def sign_slice(n, r0, r1):
                """ScalarE: sign(x rows r0..r1) -> slot0 rows 1+r0..1+r1."""
                x_v = x_tiles[n][:].rearrange("c (h w) -> c h w", h=_H)
                a_v = a_tiles[n % 2][:, 0:_SLOT].rearrange("c (h w) -> c h w", w=_WP)
                nc.scalar.activation(
                    a_v[:, 1 + r0 : 1 + r1, 1 : _W + 1],
                    x_v[:, r0:r1, :],
                    SIGN,
                    bias=nt_ap,
                )

            def sign2_slice(n, r0, r1):
                """ScalarE: slot1[r, u] = a[r, u+2] = sign(x[r-1, u+1]) — the
                shift-by-2 copy computed directly from x (keeps VectorE free
                for epilogues).  x rows r0..r1 -> slot1 rows 1+r0..1+r1,
                cols 0..54 (col 55 is the a-col-57 frame zero, pre-memset)."""
                x_v = x_tiles[n][:].rearrange("c (h w) -> c h w", h=_H)
                a1_v = a_tiles[n % 2][:, _SLOT:].rearrange("c (h w) -> c h w", w=_WP)
                nc.scalar.activation(
                    a1_v[:, 1 + r0 : 1 + r1, 0:55],
                    x_v[:, r0:r1, 1:56],
                    SIGN,
                    bias=nt_ap,
                )

            # Tiny ScalarE op: dependency "tick" separator.  Dependents of
            # the op before it release ~60ns after it instead of waiting
            # through the next full-size ACT (the sem-tick granularity
            # otherwise rounds the release up to the next op's end).
            tick_t = cpool.tile([_C, 8], bf16)

            def tick():
                nc.scalar.activation(tick_t[:], dummy[:, 0:8], IDENT)

            # Image-0 staging: signs only (residual uses the pre-biased
            # input tile directly; image 0 uses the 6-mm no-slot1 form).
            for rng0 in IMG0_RANGES:
                sign_slice(0, *rng0)
                tick()

            def stage_sign(n, ranges):
                """ScalarE staging for images 1..3.  sign2 only for the first
                half: the second half's slot1 rows come from a VectorE copy
                (stage_copy) so Scalar stays under the per-image budget."""
                (a0, a1), (b0, b1) = ranges
                sign_slice(n, a0, a1)
                sign2_slice(n, max(a0, 1), a1)
                tick()
                sign_slice(n, b0, b1)
                tick()

            def stage_copy(n):
                """VectorE: slot1 rows 29..57 = slot0 rows 29..57 shifted by
                2 cols (row 57 reads the frame zeros).  Emitted at the END of
                the previous image's pair loop so it can't head-block that
                image's epilogue stts."""
                a_t = a_tiles[n % 2]
                av = a_t[:].rearrange("c (s h w) -> c s h w", s=2, w=_WP)
                nc.vector.tensor_copy(
                    av[:, 1, 29:58, 0:56],
                    av[:, 0, 29:58, 2:58],
                )

            # PE warmup (results discarded).  ~107ns apart at mid-clock;
            # covers preamble-end -> first-deps-ready (~10.8us: sign0 after
            # the startup DGE chain).  A PE idle gap here resets the HAM
            # clock ramp, so over-covering beats under-covering.
            warm_ps = pspool.tile([_P, 2, 512], f32, name="warm_ps", tag="ps")
            for i in range(48):
                nc.tensor.matmul(
                    warm_ps[:, i % 2, :128],
                    dummy[:],
                    dummy[:],
                    start=True,
                    stop=True,
                )

            def chunk_mms(n, bank_ap, rb, use_slot1):
                """Matmuls for the chunk at a-row rb (out rows rb..rb+6),
                bank-major (tap-major pairs were tried: steady state gained
                only ~35ns/image but image-0's pair-0 then gates on sign1 —
                net loss).
                use_slot1: 5-mm form (needs slot1 copies done); else 6-mm
                form with kh=2 as three plain taps (image 0: avoids coupling
                the startup-critical first chunks to the copy pipeline)."""
                a_t = a_tiles[n % 2]
                part = tuple(a_t[:, 0:1].ap[0])
                base = int(a_t[:, 0:1].offset)

                def rhs(off, step):
                    dims = [part]
                    if step is not None:
                        dims.append((step, 2))
                    dims.append((1, _CS))
                    return bass.AP(tensor=a_t.tensor, offset=base + off, ap=dims)

                plan = [
                    (wt[:, 0:2, :], rb * _WP + 0, _WP, DR),
                    (wt[:, 2:4, :], rb * _WP + 1, _WP, DR),
                    (wt[:, 4:6, :], rb * _WP + 2, _WP, DR),
                ]
                if use_slot1:
                    plan += [
                        (wt[:, 6:8, :], (rb + 2) * _WP + 0, _SLOT, DR),
                        (wt[:, 8, :], (rb + 2) * _WP + 1, None, None),
                    ]
                else:
                    plan += [
                        (wt[:, 6, :], (rb + 2) * _WP + 0, None, None),
                        (wt[:, 8, :], (rb + 2) * _WP + 1, None, None),
                        (wt[:, 7, :], (rb + 2) * _WP + 2, None, None),
                    ]
                for i, (w_ap, off, step, pm) in enumerate(plan):
                    nc.tensor.matmul(
                        bank_ap,
                        w_ap,
                        rhs(off, step),
                        start=(i == 0),
                        stop=(i == len(plan) - 1),
                        perf_mode=pm,
                    )

            for n in range(_NPI):
                if n + 1 < _NPI:
                    # image 1's loads on sync (startup transfers still
                    # draining there; a second queue would steal SDMA
                    # engines); images 2-3 via the scalar queue.
                    load_x(n + 1, [(0, 28), (28, 56)],
                           eng=(nc.sync if n == 0 else nc.scalar))
                    stage_sign(n + 1, [(0, 28), (28, 56)])

                last_img = n == _NPI - 1

                for p in range(_NPAIR):
                    fine_tail = last_img and p == _NPAIR - 1
                    if fine_tail:
                        bank_tiles = [
                            pspool.tile([_P, 512], f32, name=f"pstb{b}", tag="ps")
                            for b in range(2)
                        ]
                        bank_aps = [bt[:, :_CS] for bt in bank_tiles]
                        bank_views = [
                            bt[:].rearrange("c (h w) -> c h w", w=_WP)
                            for bt in bank_tiles
                        ]
                    else:
                        pst = pspool.tile([_P, 2, 512], f32, name="pst", tag="ps")
                        bank_aps = [pst[:, b, :_CS] for b in range(2)]
                        pv = pst[:].rearrange("c b (h w) -> c b h w", w=_WP)
                        bank_views = [pv[:, b] for b in range(2)]
                    out_t = opool.tile([_P, 2 * _CN], bf16, name="out_t", tag="o")

                    def epi(b, h0, h1):
                        """stt halves/full of bank b into out_t: out =
                        psum*s + (x+t), the residual read straight from the
                        pre-biased input tile."""
                        nc.vector.scalar_tensor_tensor(
                            out_t[:, b * _CN + h0 * _W : b * _CN + h1 * _W],
                            bank_views[b][:, h0:h1, 0:56],
                            s_ap,
                            x_tiles[n][
                                :, (2 * p + b) * _CN + h0 * _W :
                            ][:, : (h1 - h0) * _W],
                            MULT,
                            ADD,
                        )

                    for b in range(2):
                        chunk_mms(n, bank_aps[b], (2 * p + b) * _CH,
                                  use_slot1=(n > 0))
                        if fine_tail:
                            c0 = (2 * p + b) * _CN
                            if b == 0:
                                # bank 0's store DGE rides the (idle) scalar
                                # queue so the three tail store issues don't
                                # serialize ~1.8us on sync
                                epi(b, 0, _CH)
                                nc.scalar.dma_start(
                                    o_d[n, :, c0 : c0 + _CN],
                                    out_t[:, 0:_CN],
                                )
                            else:
                                # halves on alternating queues: the two
                                # ~600ns DGE issues overlap instead of
                                # serializing after the final stt
                                for (h0, h1), eng in (
                                    ((0, 4), nc.scalar),
                                    ((4, _CH), nc.sync),
                                ):
                                    epi(b, h0, h1)
                                    eng.dma_start(
                                        o_d[n, :, c0 + h0 * _W : c0 + h1 * _W],
                                        out_t[
                                            :, _CN + h0 * _W : _CN + h1 * _W
                                        ],
                                    )
                    if not fine_tail:
                        # per-bank stts (walrus limits STT inputs to 2D/3D
                        # APs, so the two banks can't merge into one 4D op)
                        for b in range(2):
                            epi(b, 0, _CH)
                        nc.sync.dma_start(
                            o_d[n, :, p * 2 * _CN : (p + 1) * 2 * _CN],
                            out_t[:],
                        )
                        if p == _NPAIR - 1 and n + 1 < _NPI:
                            stage_copy(n + 1)

    nc.compile()
    return nc


def _get_program():
    if "nc" not in _cache:
        _cache["nc"] = _build_program()
    return _cache["nc"]


def _prep_inputs(x, weight, bias, gamma, beta, running_mean, running_var):
    # per-core batch shards
    xs = np.ascontiguousarray(
        np.asarray(x, dtype=np.float32).reshape(_NCORES, _NPI, _C, _NPIX)
    )
    # sign(weight) packed as [C, k, P] fp8, k-order =
    # [(0,0),(1,0),(0,1),(1,1),(0,2),(1,2),(2,0),(2,2),(2,1)] (kh,kw)
    wb = np.sign(np.asarray(weight, dtype=np.float32))  # [P, C, 3, 3]
    korder = [(0, 0), (1, 0), (0, 1), (1, 1), (0, 2), (1, 2), (2, 0), (2, 2), (2, 1)]
    wT = np.stack(
        [wb[:, :, kh, kw].T for kh, kw in korder], axis=1
    )  # [C, 9, P]
    wT = np.ascontiguousarray(wT).astype(ml_dtypes.float8_e4m3)
    inv = np.asarray(gamma, dtype=np.float64) / np.sqrt(
        np.asarray(running_var, dtype=np.float64) + _BN_EPS
    )
    shift = (
        np.asarray(bias, dtype=np.float64) * inv
        + np.asarray(beta, dtype=np.float64)
        - np.asarray(running_mean, dtype=np.float64) * inv
    )
    t32 = shift.astype(np.float32)
    # pre-bias the input: xp = x + t (epilogue residual); signs recover
    # sign(x) on-device via ACT bias=-t
    xs = xs + t32[None, None, :, None]
    st = np.stack([inv.astype(np.float32), -t32], axis=1)  # [P, 2] = [s, -t]
    st = np.ascontiguousarray(st)
    return [{"x": xs[i], "w": wT, "st": st} for i in range(_NCORES)]


def _run(inputs, trace=False, trace_cores=None):
    from concourse.bass_utils import run_bass_kernel_spmd

    nc = _get_program()
    in_maps = _prep_inputs(**inputs)
    res = run_bass_kernel_spmd(
        nc,
        in_maps,
        list(range(_NCORES)),
        trace=trace,
        trace_cores=trace_cores,
    )
    out = np.stack(
        [np.asarray(res.results[i]["o"]).astype(np.float32) for i in range(_NCORES)],
        axis=0,
    )
    out = out.reshape(_N, _P, _H, _W)
    return out, res


def kernel(**inputs):
    out, _ = _run(inputs, trace=False)
    return out
